# revision 2
# baseline (speedup 1.0000x reference)
"""Distributed KNN inner-product retrieval (B=256, N=500000, D=64, K=100)
as a Bass/Tile kernel on 8 TRN2 NeuronCores.

Sharding: corpus is split 8 ways along N (62500 rows/core); queries are
replicated. Per core the device:
  - streams its corpus shard in 62 groups of 1024 rows (the last group
    overlaps so every group is full; duplicates are deduped on the host)
  - PE-transposes each group into [64(d), n] layout (fp32 has no DMA
    transpose on TRN2), ScalarE copies PSUM->SBUF
  - matmuls scores with queries stationary (two 128-query chunks, packed
    into disjoint PE row-groups so each pair runs concurrently)
  - VectorE reduce_max compresses scores into 16-element block maxes
Outputs per core: bm_a/bm_b [128, 3968] block maxes (2MB total).
Host: for each query keep blocks whose max >= the 110th-largest block max
(margin over k=100 absorbs device-vs-host fp32 differences), rescore only
those ~110 blocks (~1760 rows) per query in numpy, take the exact top-k
with ties broken by lower index (matching jax.lax.top_k), then gather
corpus_id / embeddings. This is exact, not approximate: any row in the true
top-100 lives in a block whose max is >= the 100th-largest block max.
"""

from contextlib import ExitStack

import numpy as np

import concourse.bacc as bacc_mod
import concourse.tile as tile
from concourse import mybir
from concourse.bass_utils import run_bass_kernel_spmd

F32 = mybir.dt.float32

NCORE = 8
B = 256
D = 64
N_TOTAL = 500000
NLOC = N_TOTAL // NCORE          # 62500
GROUP_ROWS = 1024
NG_FULL = NLOC // GROUP_ROWS     # 61 full groups
NG = NG_FULL + 1                 # + 1 overlap group
LAST_BASE = NLOC - GROUP_ROWS    # 61476
NBLK_PER_GROUP = 64
NBLK = NG * NBLK_PER_GROUP       # 3968 16-row blocks per chunk per core


def group_base(g: int) -> int:
    return g * GROUP_ROWS if g < NG_FULL else LAST_BASE


def block_rows_table() -> np.ndarray:
    """Within-core corpus rows covered by each bm16 block, [NBLK, 16]."""
    tbl = np.empty((NBLK, 16), np.int64)
    ar = np.arange(16)
    for beta in range(NBLK):
        g, r = divmod(beta, NBLK_PER_GROUP)
        low, j = divmod(r, 32)
        t, jb = divmod(j, 8)
        tbl[beta] = group_base(g) + 8 * (16 * jb + ar) + 2 * t + low
    return tbl


def build_nc():
    nc = bacc_mod.Bacc(
        "TRN2", target_bir_lowering=False, debug=False, num_devices=NCORE
    )
    corpus = nc.dram_tensor("corpus_shard", [NLOC, D], F32, kind="ExternalInput").ap()
    qt = nc.dram_tensor("qt", [128, B], F32, kind="ExternalInput").ap()
    ident = nc.dram_tensor("ident", [128, 128], F32, kind="ExternalInput").ap()
    bm_a = nc.dram_tensor("bm_a", [128, NBLK], F32, kind="ExternalOutput").ap()
    bm_b = nc.dram_tensor("bm_b", [128, NBLK], F32, kind="ExternalOutput").ap()

    with tile.TileContext(nc) as tc, ExitStack() as ctx:
        const_pool = ctx.enter_context(tc.tile_pool(name="const", bufs=1))
        raw_pool = ctx.enter_context(tc.tile_pool(name="raw", bufs=3))
        ct_pool = ctx.enter_context(tc.tile_pool(name="ct", bufs=3))
        bm_pool = ctx.enter_context(tc.tile_pool(name="bm", bufs=1))
        ctps_pool = ctx.enter_context(tc.tile_pool(name="ctps", bufs=2, space="PSUM"))
        sps_pool = ctx.enter_context(tc.tile_pool(name="sps", bufs=3, space="PSUM"))

        qt_sb = const_pool.tile([128, B], F32)
        nc.sync.dma_start(qt_sb[:], qt)
        id_sb = const_pool.tile([128, 128], F32)
        nc.sync.dma_start(id_sb[:], ident)

        bm_sb = {
            "a": bm_pool.tile([128, NBLK], F32, name="bm_a_sb", tag="bm_a"),
            "b": bm_pool.tile([128, NBLK], F32, name="bm_b_sb", tag="bm_b"),
        }

        for g in range(NG):
            base = group_base(g)
            raw = raw_pool.tile([128, 8 * D], F32)
            nc.sync.dma_start(
                raw[:],
                corpus[base : base + GROUP_ROWS, :].rearrange(
                    "(p a) d -> p (a d)", p=128
                ),
            )

            ct_ps = ctps_pool.tile([128, 512], F32)
            for t in range(4):
                nc.tensor.transpose(
                    ct_ps[:, t * 128 : (t + 1) * 128],
                    raw[:, t * 128 : (t + 1) * 128],
                    id_sb[:],
                )
            ct = ct_pool.tile([128, 512], F32)
            nc.scalar.copy(ct[:], ct_ps[:])

            # chunk a = queries 0:128, b = 128:256; ct partitions 0:64 hold
            # even interleaved rows, 64:128 odd — packed PE row-groups. Both
            # matmuls of a chunk land in one 2-bank psum tile so the DVE
            # reduce reads 1024 elements per op (amortizes op overhead).
            for chunk, qlo in (("a", 0), ("b", 128)):
                ps = sps_pool.tile([128, 1024], F32, tag="score")
                for plo, off in ((0, 0), (64, 512)):
                    nc.tensor.matmul(
                        ps[:, off : off + 512],
                        qt_sb[plo : plo + 64, qlo : qlo + 128],
                        ct[plo : plo + 64, :],
                        start=True,
                        stop=True,
                    )
                dst = bm_sb[chunk][:, g * 64 : g * 64 + 64]
                nc.vector.reduce_max(
                    dst,
                    ps[:].rearrange("p (b s) -> p b s", s=16),
                    axis=mybir.AxisListType.X,
                )

        nc.sync.dma_start(bm_a, bm_sb["a"][:])
        nc.sync.dma_start(bm_b, bm_sb["b"][:])

    nc.compile()
    return nc


_NC_CACHE = None


def get_nc():
    global _NC_CACHE
    if _NC_CACHE is None:
        _NC_CACHE = build_nc()
    return _NC_CACHE


def run_device(query_embedding: np.ndarray, corpus: np.ndarray, trace=False):
    """Returns bm16 [B, NCORE, NBLK] block maxes plus the raw results obj."""
    qt = np.empty((128, B), np.float32)
    qt[0:64] = query_embedding.T
    qt[64:128] = query_embedding.T
    hin = {"qt": qt, "ident": np.eye(128, dtype=np.float32)}
    in_maps = [
        {
            "corpus_shard": np.ascontiguousarray(
                corpus[c * NLOC : (c + 1) * NLOC]
            ),
            **hin,
        }
        for c in range(NCORE)
    ]
    res = run_bass_kernel_spmd(
        get_nc(), in_maps, core_ids=list(range(NCORE)), trace=trace
    )
    bm16 = np.stack(
        [
            np.concatenate(
                [res.results[c]["bm_a"], res.results[c]["bm_b"]], axis=0
            )
            for c in range(NCORE)
        ],
        axis=1,
    )
    return bm16, res


def host_refine(bm16, query_embedding, corpus, corpus_id, k, cut=None):
    if cut is None:
        cut = max(110, k + 10)
    tbl = block_rows_table()
    nq = bm16.shape[0]
    bm_flat = bm16.reshape(nq, -1)
    vcut = np.partition(bm_flat, bm_flat.shape[1] - cut, axis=1)[
        :, bm_flat.shape[1] - cut
    ]
    mask = bm_flat >= vcut[:, None]
    maxblk = int(mask.sum(1).max())

    rows_pad = np.zeros((nq, maxblk, 16), np.int64)
    valid = np.zeros((nq, maxblk), bool)
    for q in range(nq):
        idx = np.nonzero(mask[q])[0]
        c, beta = np.divmod(idx, NBLK)
        rows_pad[q, : len(idx)] = tbl[beta] + (c * NLOC)[:, None]
        valid[q, : len(idx)] = True
    rows_flat = rows_pad.reshape(nq, -1)

    gat = corpus[rows_flat]
    sc = np.einsum("qmd,qd->qm", gat, query_embedding, optimize=True).astype(
        np.float32
    )
    sc[~np.repeat(valid, 16, axis=1)] = -np.inf
    # dedup rows duplicated by the overlap group
    order = np.argsort(rows_flat, axis=1, kind="stable")
    srt = np.take_along_axis(rows_flat, order, axis=1)
    dup = np.concatenate(
        [np.zeros((nq, 1), bool), srt[:, 1:] == srt[:, :-1]], axis=1
    )
    for q in range(nq):
        sc[q][order[q][dup[q]]] = -np.inf

    out_idx = np.empty((nq, k), np.int64)
    for q in range(nq):
        o = np.lexsort((rows_flat[q], -sc[q]))[:k]
        out_idx[q] = rows_flat[q][o]

    item_ids = corpus_id[out_idx]
    scores = np.einsum("qkd,qd->qk", corpus[out_idx], query_embedding).astype(
        np.float32
    )
    embeddings = corpus[out_idx]
    return item_ids, scores, embeddings


def kernel(query_embedding, corpus, corpus_id, num_items):
    query_embedding = np.ascontiguousarray(
        np.asarray(query_embedding, dtype=np.float32)
    )
    corpus = np.ascontiguousarray(np.asarray(corpus, dtype=np.float32))
    corpus_id = np.asarray(corpus_id)
    k = int(num_items)

    bm16, _ = run_device(query_embedding, corpus)
    return host_refine(bm16, query_embedding, corpus, corpus_id, k)
